# revision 9
# baseline (speedup 1.0000x reference)
"""Trainium2 Bass kernel for a binarized ResNet BasicBlock (stride-2), v2.

Reference computation (per image):
    residual = BN2(conv1x1(avgpool2x2(x), w_ds))          # full precision
    body     = BN1(conv3x3_s2_p1(sign(x), sign(w_body)))  # binarized
    out      = body + residual

Shapes: x [16, 32, 224, 224] f32 -> out [16, 64, 112, 112] f32.
Sharding: data-parallel over batch, 2 images per core on 8 cores.

v2 design (vs the v1 per-tap K=64/K=32 scheme): pack the full 3x1 column
of conv taps into K=96 matmuls via a 3-sub-row S layout, and the whole
residual into one K=64 matmul per tile via a DVE column-pair pre-sum.
All data is fp8e4m3 (sign-exact for the body; the residual branch is
~2% of output magnitude so fp8 quantization is negligible).

Per chunk pair (2 chunks x 16 output rows):
  * Two half-DMAs (gpsimd + scalar rings) load zz[pair] -> V [128, 17, 224]
    fp8, partition 64q+par*32+ci, slot s = input rows 2*(16c-1+s)+par.
  * One 128-part DVE tensor_tensor: Vc[p, j, X] = V[p, j+1, 2X] + V[p, j+1, 2X+1]
    (the avgpool column pre-sum; the row sum happens in the residual matmul K).
  * Per chunk, two DVE tensor_scalar ops build T [96, 16, 228] fp8:
    partitions 0:32 = sign(row 2Y) (ky=1), 32:64 = sign(row 2Y+1) (ky=2),
    64:96 = sign(row 2Y-1) (ky=0, copied one slot back), via the u16-pair
    bit trick (v & 0x8080) | 0x3838. Columns 0:2 are zero pads; tap kx
    reads positions kx+1 + 2X.
  * Per 4-output-row tile and chunk: 3 body matmuls K=96 (one per kx) and
    1 residual matmul K=64 accumulate into one PSUM bank; chunk A uses PE
    columns 0:64, chunk B 64:128 so the pair's matmuls can overlap in the
    array. One 128-partition ScalarE activation applies both BNs while
    evacuating PSUM -> SBUF bf16; one DMA stores the pair (host upcasts).
"""

import numpy as np
import ml_dtypes

EPS = 1e-5

B, CIN, COUT, H, W = 16, 32, 64, 224, 224
N_CORES = 8
B_CORE = B // N_CORES  # 2 images per core

NP_FP8 = ml_dtypes.float8_e4m3
NP_BF16 = ml_dtypes.bfloat16


def build_nc(b_core=B_CORE, cin=CIN, cout=COUT, h=H, w=W, chunk_rows=16,
             loop_reps=1, ablate=None, mm_order="kxt"):
    from contextlib import nullcontext
    import concourse.bass as bass
    import concourse.bacc as bacc
    import concourse.mybir as mybir
    import concourse.tile as tile

    ho, wo = h // 2, w // 2
    n_chunks = ho // chunk_rows
    T = chunk_rows // 4  # matmul tiles per chunk (4 output rows each)
    G = b_core * n_chunks
    assert G % 2 == 0
    n_pairs = G // 2
    ns = chunk_rows + 1  # V slots per chunk

    f32 = mybir.dt.float32
    bf16 = mybir.dt.bfloat16
    fp8 = mybir.dt.float8e4
    u16 = mybir.dt.uint16

    nc = bacc.Bacc("TRN2", target_bir_lowering=False, debug=False)

    zz = nc.dram_tensor("zz", [n_pairs, 128, ns, w], fp8, kind="ExternalInput")
    # Body tap weights, [96, 3, 64]: partition sub*32+ci (sub0=ky1, sub1=ky2,
    # sub2=ky0 to match T), free = (kx, cout).
    wk_d = nc.dram_tensor("wk", [3 * cin, 3, cout], fp8, kind="ExternalInput")
    wr_d = nc.dram_tensor("wr", [4 * cin, cout], fp8, kind="ExternalInput")
    sc_d = nc.dram_tensor("bn_scale", [2 * cout, 1], f32, kind="ExternalInput")
    bi_d = nc.dram_tensor("bn_bias", [2 * cout, 1], f32, kind="ExternalInput")
    out_d = nc.dram_tensor("out", [n_pairs, 128, chunk_rows, wo], bf16,
                           kind="ExternalOutput")

    with tile.TileContext(nc) as tc:
        with tc.tile_pool(name="consts", bufs=1) as cpool:
            wk = cpool.tile([3 * cin, 3, cout], fp8)
            nc.sync.dma_start(out=wk[:, :, :], in_=wk_d.ap()[:, :, :])
            wr = cpool.tile([4 * cin, cout], fp8)
            nc.sync.dma_start(out=wr[:, :], in_=wr_d.ap()[:, :])
            sc = cpool.tile([2 * cout, 1], f32)
            nc.sync.dma_start(out=sc[:, :], in_=sc_d.ap()[:, :])
            bi = cpool.tile([2 * cout, 1], f32)
            nc.sync.dma_start(out=bi[:, :], in_=bi_d.ap()[:, :])

            with (
                tc.tile_pool(name="vpool", bufs=3) as vpool,
                tc.tile_pool(name="vcpool", bufs=3) as vcpool,
                tc.tile_pool(name="spool", bufs=1) as spool,
                tc.tile_pool(name="opool", bufs=3) as opool,
                tc.tile_pool(name="pspool", bufs=2, space="PSUM") as pspool,
            ):
                # T buffers are managed manually (not pool-cycled) so their
                # zero-pad columns 0:2 are initialized exactly once. Only the
                # two buffers pair 0 uses are padded up front; the rest are
                # padded after pair 0's sign ops so the DVE reaches pair 0's
                # work sooner (shorter pipeline fill).
                n_tbufs = 6
                t_bufs = []
                for si in range(n_tbufs):
                    tb = spool.tile([3 * cin, chunk_rows, w + 4], fp8,
                                    name=f"tbuf{si}")
                    if si < 2:
                        nc.vector.memset(tb[:, :, 0:2], 0.0)
                    t_bufs.append(tb)

                reps_ctx = (
                    tc.For_i(0, loop_reps, 1) if loop_reps > 1 else nullcontext()
                )
                with reps_ctx:
                  for pair in range(n_pairs):
                    v = vpool.tile([128, ns, w], fp8)
                    vc = vcpool.tile([128, chunk_rows, wo], fp8)
                    o = opool.tile([128, chunk_rows, wo], bf16)
                    ps = pspool.tile([128, T, 512], f32, tag="ps")
                    if ablate != "no_in":
                        # per-chunk halves on two rings: chunk A's signs can
                        # start after half the transfer
                        nc.gpsimd.dma_start(out=v[0:64, :, :],
                                            in_=zz.ap()[pair, 0:64, :, :])
                        nc.scalar.dma_start(out=v[64:128, :, :],
                                            in_=zz.ap()[pair, 64:128, :, :])
                    ts = []
                    for q in range(2):
                        g = 2 * pair + q
                        c = g % n_chunks
                        tbuf = t_bufs[g % n_tbufs]
                        ts.append(tbuf)
                        if ablate != "no_in":
                            pv = 64 * q
                            # sub1+sub2: sign of rows 2Y, 2Y+1 (slot j+1)
                            nc.vector.tensor_scalar(
                                tbuf.bitcast(u16)[0:64, :, 1 : 1 + wo],
                                v.bitcast(u16)[pv : pv + 64, 1:ns, 0:wo],
                                0x8080, 0x3838,
                                mybir.AluOpType.bitwise_and,
                                mybir.AluOpType.bitwise_or,
                            )
                            # sub0 (sign of row 2Y-1 = par1 one slot back):
                            # slots 1:16 are copies of already-computed par1
                            # signs, done as an SBUF->SBUF DMA on the sync
                            # ring to keep the DVE off the critical path;
                            # only slot 0 (the halo row) needs a fresh sign.
                            if pair == 0:
                                # pair 0: DVE shifted sign for sub0 — the
                                # DMA-copy's sem round trip would sit on the
                                # pipeline-fill critical path
                                nc.vector.tensor_scalar(
                                    tbuf.bitcast(u16)[64:96, :, 1 : 1 + wo],
                                    v.bitcast(u16)[pv + 32 : pv + 64, 0 : ns - 1, 0:wo],
                                    0x8080, 0x3838,
                                    mybir.AluOpType.bitwise_and,
                                    mybir.AluOpType.bitwise_or,
                                )
                            else:
                                nc.vector.tensor_scalar(
                                    tbuf.bitcast(u16)[64:96, 0:1, 1 : 1 + wo],
                                    v.bitcast(u16)[pv + 32 : pv + 64, 0:1, 0:wo],
                                    0x8080, 0x3838,
                                    mybir.AluOpType.bitwise_and,
                                    mybir.AluOpType.bitwise_or,
                                )
                                nc.sync.dma_start(
                                    out=tbuf[64:96, 1:chunk_rows, :],
                                    in_=tbuf[32:64, 0 : chunk_rows - 1, :],
                                )
                            if c == 0:
                                # output row 0 reads input row -1: zero, not
                                # sign(0)
                                nc.vector.memset(tbuf[64:96, 0:1, :], 0.0)
                    if ablate != "no_in":
                        # residual column pre-sum (both chunks at once);
                        # emitted after the signs so the body matmuls can
                        # start as soon as the signs land (Vc is first
                        # needed by the residual matmul, ~1.3us later).
                        nc.vector.tensor_tensor(
                            vc[:, :, :],
                            v[:, 1:ns, 0 : w : 2],
                            v[:, 1:ns, 1 : w : 2],
                            mybir.AluOpType.add,
                        )
                    if pair == 0:
                        for tb in t_bufs[2:]:
                            nc.vector.memset(tb[:, :, 0:2], 0.0)
                    if ablate != "io_only":
                        if mm_order == "kx_outer":
                            # One weight set per (q, kx): LDWEIGHTS amortizes
                            # over the 4 t-tiles and hides under matmuls.
                            for q in range(2):
                                pc = 64 * q
                                for kx in range(3):
                                    cols = slice(kx + 1, kx + 1 + 2 * wo, 2)
                                    for t in range(T):
                                        j0 = 4 * t
                                        nc.tensor.matmul(
                                            ps[pc : pc + 64, t, 0 : 4 * wo],
                                            wk[:, kx, :],
                                            ts[q][:, j0 : j0 + 4, cols],
                                            start=(kx == 0), stop=False,
                                            tile_position=(0, pc),
                                        )
                                for t in range(T):
                                    j0 = 4 * t
                                    nc.tensor.matmul(
                                        ps[pc : pc + 64, t, 0 : 4 * wo],
                                        wr[2 * cin * q : 2 * cin * (q + 1), :],
                                        vc[pc : pc + 64, j0 : j0 + 4, :],
                                        start=False, stop=True,
                                        tile_position=(pc, pc),
                                    )
                        elif mm_order == "kxt":
                            # Weight set changes only at kx boundaries (the
                            # 8 inner matmuls per kx reuse the loaded cells),
                            # while consecutive matmuls still alternate PSUM
                            # partition halves (same-region back-to-back
                            # accumulation serializes with full drains).
                            for kx in range(3):
                                cols = slice(kx + 1, kx + 1 + 2 * wo, 2)
                                for t in range(T):
                                    j0 = 4 * t
                                    for q in range(2):
                                        pc = 64 * q
                                        nc.tensor.matmul(
                                            ps[pc : pc + 64, t, 0 : 4 * wo],
                                            wk[:, kx, :],
                                            ts[q][:, j0 : j0 + 4, cols],
                                            start=(kx == 0), stop=False,
                                            tile_position=(0, pc),
                                        )
                            for t in range(T):
                                j0 = 4 * t
                                for q in range(2):
                                    pc = 64 * q
                                    nc.tensor.matmul(
                                        ps[pc : pc + 64, t, 0 : 4 * wo],
                                        wr[2 * cin * q : 2 * cin * (q + 1), :],
                                        vc[pc : pc + 64, j0 : j0 + 4, :],
                                        start=False, stop=True,
                                        tile_position=(pc, pc),
                                    )
                        elif mm_order == "q_outer":
                            # Chunk A's matmuls all precede chunk B's, so
                            # ACT(A) + store(A) overlap B's matmuls and the
                            # pair tail shrinks to one half-evacuation.
                            for q in range(2):
                                pc = 64 * q
                                for t in range(T):
                                    j0 = 4 * t
                                    for kx in range(3):
                                        cols = slice(kx + 1, kx + 1 + 2 * wo, 2)
                                        nc.tensor.matmul(
                                            ps[pc : pc + 64, t, 0 : 4 * wo],
                                            wk[:, kx, :],
                                            ts[q][:, j0 : j0 + 4, cols],
                                            start=(kx == 0), stop=False,
                                            tile_position=(0, pc),
                                        )
                                    nc.tensor.matmul(
                                        ps[pc : pc + 64, t, 0 : 4 * wo],
                                        wr[2 * cin * q : 2 * cin * (q + 1), :],
                                        vc[pc : pc + 64, j0 : j0 + 4, :],
                                        start=False, stop=True,
                                        tile_position=(pc, pc),
                                    )
                                nc.scalar.activation(
                                    o[pc : pc + 64].rearrange(
                                        "p (t j) x -> p t (j x)", t=T),
                                    ps[pc : pc + 64, :, 0 : 4 * wo],
                                    mybir.ActivationFunctionType.Identity,
                                    bias=bi[pc : pc + 64, :],
                                    scale=sc[pc : pc + 64, :],
                                )
                                out_eng = nc.sync if q == 0 else nc.gpsimd
                                out_eng.dma_start(
                                    out=out_d.ap()[pair, pc : pc + 64, :, :],
                                    in_=o[pc : pc + 64, :, :],
                                )
                        else:
                            for t in range(T):
                                j0 = 4 * t
                                for kx in range(3):
                                    cols = slice(kx + 1, kx + 1 + 2 * wo, 2)
                                    for q in range(2):
                                        pc = 64 * q
                                        nc.tensor.matmul(
                                            ps[pc : pc + 64, t, 0 : 4 * wo],
                                            wk[:, kx, :],
                                            ts[q][:, j0 : j0 + 4, cols],
                                            start=(kx == 0), stop=False,
                                            tile_position=(0, pc),
                                        )
                                for q in range(2):
                                    pc = 64 * q
                                    nc.tensor.matmul(
                                        ps[pc : pc + 64, t, 0 : 4 * wo],
                                        wr[2 * cin * q : 2 * cin * (q + 1), :],
                                        vc[pc : pc + 64, j0 : j0 + 4, :],
                                        start=False, stop=True,
                                        tile_position=(pc, pc),
                                    )
                        if mm_order != "q_outer":
                            nc.scalar.activation(
                                o.rearrange("p (t j) x -> p t (j x)", t=T),
                                ps[:, :, 0 : 4 * wo],
                                mybir.ActivationFunctionType.Identity,
                                bias=bi[:, :],
                                scale=sc[:, :],
                            )
                            nc.gpsimd.dma_start(
                                out=out_d.ap()[pair, :, :, :], in_=o[:, :, :]
                            )
    nc.compile()
    return nc


def prep_weights(w_body, w_ds, bn1_gamma, bn1_beta, bn1_mean, bn1_var,
                 bn2_gamma, bn2_beta, bn2_mean, bn2_var):
    """Host-side parameter folding (all small tensors)."""
    cout, cin = w_body.shape[0], w_body.shape[1]
    inv1 = (bn1_gamma / np.sqrt(bn1_var + EPS)).astype(np.float32)
    inv2 = (bn2_gamma / np.sqrt(bn2_var + EPS)).astype(np.float32)
    shift1 = (bn1_beta - bn1_mean * inv1).astype(np.float32)
    shift2 = (bn2_beta - bn2_mean * inv2).astype(np.float32)

    wb_sign = np.where(w_body >= 0, 1.0, -1.0).astype(np.float32)  # [co,ci,ky,kx]

    # Body taps [96, 3, 64]: partitions (sub, ci) with sub0=ky1, sub1=ky2,
    # sub2=ky0; free = (kx, co).
    wk = np.empty((3 * cin, 3, cout), dtype=np.float32)
    for sub, ky in enumerate((1, 2, 0)):
        # [co, ci, kx] -> [ci, kx, co]
        wk[sub * cin : (sub + 1) * cin] = wb_sign[:, :, ky, :].transpose(1, 2, 0)

    # Residual: w_ds folded with BN2 and divided by BN1 scale (the final
    # activation multiplies by inv1); 1/4 is the avgpool mean.
    wrb = (w_ds[:, :, 0, 0] * (inv2 / (4.0 * inv1))[:, None]).T  # [ci, co]
    wr = np.tile(wrb, (4, 1))  # [(q par ci), co]

    return dict(
        wk=wk.astype(NP_FP8),
        wr=wr.astype(NP_FP8),
        bn_scale=np.tile(inv1, 2).reshape(2 * cout, 1),
        bn_bias=np.tile(shift1 + shift2, 2).reshape(2 * cout, 1),
    )


def make_zz(x8, cin=CIN, h=H, w=W, chunk_rows=16):
    """Host layout prep: per-chunk-pair DMA payloads from fp8 input.

    x8: [b_core, ci, r, u] fp8. Returns zz[pair, 64q+par*32+ci, s, u] =
    x[b, ci, 32c - 2 + 2s + par, u] for chunk g = 2*pair + q (b = g//7,
    c = g%7), with out-of-range rows zero.
    """
    if x8.dtype != NP_FP8:
        x8 = np.asarray(x8, np.float32).astype(NP_FP8)
    b_core = x8.shape[0]
    hh = h // 2
    n_chunks = hh // chunk_rows
    ns = chunk_rows + 1
    G = b_core * n_chunks
    xp = np.zeros((b_core, cin, h + 2, w), NP_FP8)
    xp[:, :, 2:, :] = x8
    zz = np.empty((G // 2, 128, ns, w), NP_FP8)
    for g in range(G):
        b, c = divmod(g, n_chunks)
        # [ci, 34, w] -> [ci, 17, 2, w] -> [2, ci, 17, w] -> [64, 17, w]
        blk = xp[b, :, 32 * c : 32 * c + 2 * ns, :]
        blk = blk.reshape(cin, ns, 2, w).transpose(2, 0, 1, 3).reshape(
            2 * cin, ns, w)
        zz[g // 2, 64 * (g % 2) : 64 * (g % 2) + 64] = blk
    return zz


def unpack_out(res_out, b_core=B_CORE, cout=COUT, ho=H // 2, wo=W // 2,
               chunk_rows=16):
    """res_out: [n_pairs, 128, 16, 112] bf16 -> [b_core, cout, ho, wo] f32."""
    n_chunks = ho // chunk_rows
    y = np.empty((b_core, cout, ho, wo), np.float32)
    G = b_core * n_chunks
    for g in range(G):
        b, c = divmod(g, n_chunks)
        q = g % 2
        y[b, :, 16 * c : 16 * c + 16, :] = res_out[
            g // 2, 64 * q : 64 * q + 64].astype(np.float32)
    return y


def kernel(x, w_body, bn1_gamma, bn1_beta, bn1_mean, bn1_var,
           w_ds, bn2_gamma, bn2_beta, bn2_mean, bn2_var):
    from concourse.bass_utils import run_bass_kernel_spmd

    x8 = np.asarray(x, dtype=np.float32).astype(NP_FP8)
    params = prep_weights(
        np.asarray(w_body, np.float32), np.asarray(w_ds, np.float32),
        np.asarray(bn1_gamma, np.float32), np.asarray(bn1_beta, np.float32),
        np.asarray(bn1_mean, np.float32), np.asarray(bn1_var, np.float32),
        np.asarray(bn2_gamma, np.float32), np.asarray(bn2_beta, np.float32),
        np.asarray(bn2_mean, np.float32), np.asarray(bn2_var, np.float32),
    )

    nc = build_nc()
    in_maps = [
        {"zz": make_zz(x8[k * B_CORE : (k + 1) * B_CORE]), **params}
        for k in range(N_CORES)
    ]
    res = run_bass_kernel_spmd(nc, in_maps, core_ids=list(range(N_CORES)))
    return np.concatenate([unpack_out(r["out"]) for r in res.results], axis=0)
